# revision 1
# baseline (speedup 1.0000x reference)
"""CrossModalTemporalAligner kernel for Trainium2 (8 NeuronCores, Bass/Tile).

Math (per batch b, node n):
    Q = H_i[b,:,n,:] @ Wq.T + bq            [Ti, d]
    K = H_j[b,:,n,:] @ Wk.T + bk            [Tj, d]
    V = H_j[b,:,n,:] @ Wv.T + bv            [Tj, d]
    S = Q @ K.T / (sqrt(d) * tau)           [Ti, Tj]
    P = softmax(S + log(exp(-gamma*dist) + 1e-8), axis=-1)
    O = P @ V                               [Ti, d]

Device strategy: data-parallel over the node axis (64 nodes -> 8 nodes/core),
each (b, n) pair fully independent.  The softmax+decay is computed in the
equivalent multiplicative form  P ~ exp(S) * (exp(-gamma*dist)+1e-8)  and
normalized by the row sum (scores are O(6) for these inputs so max-free exp is
safe in fp32).  1/(sqrt(d)*tau) is folded into Wq host-side; the decay factor
matrix is precomputed host-side from the scalar log_gamma.

On-device layout per pair (P=128 partitions, all fp32):
    X_i, X_j loaded naturally [t, d], transposed on PE -> XiT/XjT [d, t]
    QT[oc] = (Wq')  XiT   [dq, t]    lhsT = WqT chunk  (host-pretransposed)
    KT[oc] = (Wk )  XjT   [dk, s]
    V[sc]  = X_j Wv.T     [s, dv]    lhsT = XjT block
    ST[sc] = KT.T-contracted with QT  -> [s_block, t]  (= S transposed)
    PT[sc] = exp(ST) * Dmat[sc]      (ACT exp, DVE multiply)
    rowsum[1, t] += ones.T @ PT[sc]  (PE), transposed to [t,1] blocks on PE,
    reciprocal on DVE; O[tb] = PT-contracted with V, scaled by recip at evict.
"""

import time

import numpy as np

B, T, NNODES, D = 4, 512, 64, 512
NCORES = 8
NL = NNODES // NCORES  # nodes per core
P = 128
C4 = 4  # 512 / 128

_CACHE = {}


def _build_program(with_bq, with_bk, with_bv):
    import concourse.bass as bass
    import concourse.mybir as mybir
    from concourse import bacc
    from concourse.bass import ts
    from concourse.masks import make_identity
    from concourse.tile import TileContext

    f32 = mybir.dt.float32
    AF = mybir.ActivationFunctionType
    ALU = mybir.AluOpType

    nc = bacc.Bacc(
        "TRN2", num_devices=NCORES, debug=False, target_bir_lowering=False
    )
    hi = nc.dram_tensor("H_i", [B, T, NL, D], f32, kind="ExternalInput").ap()
    hj = nc.dram_tensor("H_j", [B, T, NL, D], f32, kind="ExternalInput").ap()
    wqT = nc.dram_tensor("WqT", [D, D], f32, kind="ExternalInput").ap()
    wkT = nc.dram_tensor("WkT", [D, D], f32, kind="ExternalInput").ap()
    wvT = nc.dram_tensor("WvT", [D, D], f32, kind="ExternalInput").ap()
    dmat = nc.dram_tensor("Dmat", [T, T], f32, kind="ExternalInput").ap()
    bq = bk = bv = None
    if with_bq:
        bq = nc.dram_tensor("bq", [1, D], f32, kind="ExternalInput").ap()
    if with_bk:
        bk = nc.dram_tensor("bk", [1, D], f32, kind="ExternalInput").ap()
    if with_bv:
        bv = nc.dram_tensor("bv", [1, D], f32, kind="ExternalInput").ap()
    out = nc.dram_tensor("Out", [B, T, NL, D], f32, kind="ExternalOutput").ap()

    with TileContext(nc) as tc:
        with (
            tc.tile_pool(name="const", bufs=1) as cpool,
            tc.tile_pool(name="xin", bufs=2) as xpool,
            tc.tile_pool(name="xt", bufs=2) as xtpool,
            tc.tile_pool(name="proj", bufs=2) as projpool,
            tc.tile_pool(name="pmat", bufs=2) as ppool,
            tc.tile_pool(name="outs", bufs=3) as opool,
            tc.tile_pool(name="small", bufs=2) as spool,
            tc.tile_pool(name="psum", bufs=6, space="PSUM") as psum,
            tc.tile_pool(name="psum_s", bufs=2, space="PSUM") as psum_s,
        ):
            # ---- constants ----
            wq_sb = cpool.tile([P, C4, D], f32, name="wq_sb")
            nc.sync.dma_start(out=wq_sb[:], in_=wqT.rearrange("(c p) n -> p c n", p=P))
            wk_sb = cpool.tile([P, C4, D], f32, name="wk_sb")
            nc.sync.dma_start(out=wk_sb[:], in_=wkT.rearrange("(c p) n -> p c n", p=P))
            wv_sb = cpool.tile([P, C4, D], f32, name="wv_sb")
            nc.sync.dma_start(out=wv_sb[:], in_=wvT.rearrange("(c p) n -> p c n", p=P))
            dm_sb = cpool.tile([P, C4, T], f32, name="dm_sb")
            nc.sync.dma_start(out=dm_sb[:], in_=dmat.rearrange("(c p) n -> p c n", p=P))
            identity = cpool.tile([P, P], f32, name="identity")
            make_identity(nc, identity[:])
            ones_col = cpool.tile([P, 1], f32, name="ones_col")
            nc.gpsimd.memset(ones_col[:], 1.0)
            ones_row = None
            if with_bq or with_bk or with_bv:
                ones_row = cpool.tile([1, T], f32, name="ones_row")
                nc.gpsimd.memset(ones_row[:], 1.0)
            bq_sb = bk_sb = bv_sb = None
            if with_bq:
                bq_sb = cpool.tile([1, D], f32, name="bq_sb")
                nc.sync.dma_start(out=bq_sb[:], in_=bq[:])
            if with_bk:
                bk_sb = cpool.tile([1, D], f32, name="bk_sb")
                nc.sync.dma_start(out=bk_sb[:], in_=bk[:])
            if with_bv:
                bv_sb = cpool.tile([1, D], f32, name="bv_sb")
                nc.sync.dma_start(out=bv_sb[:], in_=bv[:])

            for b in range(B):
                for nl in range(NL):
                    # ---- load activations, naturally [t, d] ----
                    xi = xpool.tile([P, C4, D], f32, tag="xi", name="xi")
                    nc.sync.dma_start(
                        out=xi[:], in_=hi[b, :, nl, :].rearrange("(c p) d -> p c d", p=P)
                    )
                    xj = xpool.tile([P, C4, D], f32, tag="xj", name="xj")
                    nc.sync.dma_start(
                        out=xj[:], in_=hj[b, :, nl, :].rearrange("(c p) d -> p c d", p=P)
                    )

                    # ---- transpose to [d, t] on PE ----
                    xiT = xtpool.tile([P, C4, T], f32, tag="xiT", name="xiT")
                    xjT = xtpool.tile([P, C4, T], f32, tag="xjT", name="xjT")
                    for src, dst, on_act in ((xi, xiT, True), (xj, xjT, False)):
                        for dc in range(C4):
                            pt = psum.tile([P, T], f32, tag="mm", name="pt")
                            for tcc in range(C4):
                                nc.tensor.transpose(
                                    pt[:, ts(tcc, P)], src[:, tcc, ts(dc, P)], identity[:]
                                )
                            if on_act:
                                nc.scalar.copy(dst[:, dc, :], pt[:])
                            else:
                                nc.vector.tensor_copy(dst[:, dc, :], pt[:])

                    # ---- projections ----
                    qT = projpool.tile([P, C4, T], f32, tag="qT", name="qT")
                    for oc in range(C4):
                        pq = psum.tile([P, T], f32, tag="mm", name="pq")
                        for kc in range(C4):
                            nc.tensor.matmul(
                                pq[:],
                                wq_sb[:, kc, ts(oc, P)],
                                xiT[:, kc, :],
                                start=(kc == 0),
                                stop=(kc == 3 and not with_bq),
                            )
                        if with_bq:
                            nc.tensor.matmul(
                                pq[:], bq_sb[0:1, ts(oc, P)], ones_row[0:1, :],
                                start=False, stop=True,
                            )
                        nc.scalar.copy(qT[:, oc, :], pq[:])

                    kT = projpool.tile([P, C4, T], f32, tag="kT", name="kT")
                    for oc in range(C4):
                        pk = psum.tile([P, T], f32, tag="mm", name="pk")
                        for kc in range(C4):
                            nc.tensor.matmul(
                                pk[:],
                                wk_sb[:, kc, ts(oc, P)],
                                xjT[:, kc, :],
                                start=(kc == 0),
                                stop=(kc == 3 and not with_bk),
                            )
                        if with_bk:
                            nc.tensor.matmul(
                                pk[:], bk_sb[0:1, ts(oc, P)], ones_row[0:1, :],
                                start=False, stop=True,
                            )
                        nc.scalar.copy(kT[:, oc, :], pk[:])

                    vm = projpool.tile([P, C4, D], f32, tag="vm", name="vm")
                    for sc in range(C4):
                        pv = psum.tile([P, D], f32, tag="mm", name="pv")
                        for kc in range(C4):
                            nc.tensor.matmul(
                                pv[:],
                                xjT[:, kc, ts(sc, P)],
                                wv_sb[:, kc, :],
                                start=(kc == 0),
                                stop=(kc == 3 and not with_bv),
                            )
                        if with_bv:
                            nc.tensor.matmul(
                                pv[:], ones_row[0:1, 0:P], bv_sb[0:1, :],
                                start=False, stop=True,
                            )
                        nc.vector.tensor_copy(vm[:, sc, :], pv[:])

                    # ---- S^T per s-block, multiplicative-decay softmax ----
                    pm = ppool.tile([P, C4, T], f32, tag="pm", name="pm")
                    prow = psum_s.tile([1, T], f32, tag="sm", name="prow")
                    for sc in range(C4):
                        ps = psum.tile([P, T], f32, tag="mm", name="ps")
                        for qc in range(C4):
                            nc.tensor.matmul(
                                ps[:],
                                kT[:, qc, ts(sc, P)],
                                qT[:, qc, :],
                                start=(qc == 0),
                                stop=(qc == 3),
                            )
                        nc.scalar.activation(pm[:, sc, :], ps[:], AF.Exp)
                        nc.vector.tensor_tensor(
                            pm[:, sc, :], pm[:, sc, :], dm_sb[:, sc, :], ALU.mult
                        )
                        nc.tensor.matmul(
                            prow[:], ones_col[:], pm[:, sc, :],
                            start=(sc == 0), stop=(sc == 3),
                        )

                    rsum_row = spool.tile([1, T], f32, tag="rsr", name="rsum_row")
                    nc.scalar.copy(rsum_row[:], prow[:])
                    rr_ps = psum_s.tile([P, C4], f32, tag="sm", name="rr_ps")
                    for tb in range(C4):
                        nc.tensor.transpose(
                            rr_ps[:, tb : tb + 1],
                            rsum_row[0:1, ts(tb, P)],
                            identity[0:1, 0:1],
                        )
                    rr_col = spool.tile([P, C4], f32, tag="rrc", name="rr_col")
                    nc.vector.reciprocal(rr_col[:], rr_ps[:])

                    # ---- O = P V, normalized at eviction ----
                    for tb in range(C4):
                        po = psum.tile([P, D], f32, tag="mm", name="po")
                        for sc in range(C4):
                            nc.tensor.matmul(
                                po[:],
                                pm[:, sc, ts(tb, P)],
                                vm[:, sc, :],
                                start=(sc == 0),
                                stop=(sc == 3),
                            )
                        ob = opool.tile([P, D], f32, tag="ob", name="ob")
                        nc.vector.tensor_scalar_mul(ob[:], po[:], rr_col[:, tb : tb + 1])
                        nc.sync.dma_start(
                            out=out[b, ts(tb, P), nl, :], in_=ob[:]
                        )

    nc.finalize()
    return nc


def _get_runner(with_bq, with_bk, with_bv):
    """Build (once) the Bass program and a jit-compiled 8-core executor.

    Replicates the multi-core body of concourse.bass2jax.run_bass_via_pjrt so
    the jax.jit executable is cached across calls (run_bass_via_pjrt builds a
    fresh jit per invocation).
    """
    key = (with_bq, with_bk, with_bv)
    if key in _CACHE:
        return _CACHE[key]

    import jax
    import concourse.mybir as mybir
    from concourse import bass2jax
    from jax.sharding import Mesh, PartitionSpec
    from jax.experimental.shard_map import shard_map

    nc = _build_program(with_bq, with_bk, with_bv)
    bass2jax.install_neuronx_cc_hook()

    partition_name = nc.partition_id_tensor.name if nc.partition_id_tensor else None
    in_names, out_names, out_avals, zero_outs = [], [], [], []
    for alloc in nc.m.functions[0].allocations:
        if not isinstance(alloc, mybir.MemoryLocationSet):
            continue
        name = alloc.memorylocations[0].name
        if alloc.kind == "ExternalInput":
            if name != partition_name:
                in_names.append(name)
        elif alloc.kind == "ExternalOutput":
            out_names.append(name)
            shape = tuple(alloc.tensor_shape)
            dtype = mybir.dt.np(alloc.dtype)
            out_avals.append(jax.core.ShapedArray(shape, dtype))
            zero_outs.append(np.zeros(shape, dtype))
    n_params = len(in_names)
    n_outs = len(out_avals)
    in_names = in_names + out_names
    if partition_name is not None:
        in_names.append(partition_name)

    donate = tuple(range(n_params, n_params + n_outs))

    def _body(*args):
        operands = list(args)
        if partition_name is not None:
            operands.append(bass2jax.partition_id_tensor())
        outs = bass2jax._bass_exec_p.bind(
            *operands,
            out_avals=tuple(out_avals),
            in_names=tuple(in_names),
            out_names=tuple(out_names),
            lowering_input_output_aliases=(),
            sim_require_finite=True,
            sim_require_nnan=True,
            nc=nc,
        )
        return tuple(outs)

    devices = jax.devices()[:NCORES]
    mesh = Mesh(np.asarray(devices), ("core",))
    in_specs = (PartitionSpec("core"),) * (n_params + n_outs)
    out_specs = (PartitionSpec("core"),) * len(out_names)
    sharded = jax.jit(
        shard_map(_body, mesh=mesh, in_specs=in_specs, out_specs=out_specs,
                  check_rep=False),
        donate_argnums=donate,
        keep_unused=True,
    )
    param_names = in_names[:n_params]

    def run(in_maps, timers=None):
        concat_in = [
            np.concatenate([np.asarray(m[name]) for m in in_maps], axis=0)
            for name in param_names
        ]
        concat_zeros = [
            np.zeros((NCORES * z.shape[0], *z.shape[1:]), z.dtype) for z in zero_outs
        ]
        if timers is not None:
            t0 = time.perf_counter()
            out_arrs = sharded(*concat_in, *concat_zeros)
            jax.block_until_ready(out_arrs)
            timers.append(time.perf_counter() - t0)
        else:
            out_arrs = sharded(*concat_in, *concat_zeros)
        full = np.asarray(out_arrs[0]).reshape(NCORES, *out_avals[0].shape)
        return full

    _CACHE[key] = run
    return run


def _prepare_in_maps(H_i, H_j, Wq, bq, Wk, bk, Wv, bv, log_gamma, log_tau):
    H_i = np.asarray(H_i, dtype=np.float32)
    H_j = np.asarray(H_j, dtype=np.float32)
    Wq = np.asarray(Wq, dtype=np.float32)
    Wk = np.asarray(Wk, dtype=np.float32)
    Wv = np.asarray(Wv, dtype=np.float32)
    bq = np.asarray(bq, dtype=np.float32)
    bk = np.asarray(bk, dtype=np.float32)
    bv = np.asarray(bv, dtype=np.float32)
    lg = np.float32(np.asarray(log_gamma))
    lt = np.float32(np.asarray(log_tau))

    tau = np.maximum(np.exp(lt, dtype=np.float32), np.float32(0.01))
    gamma = np.maximum(np.exp(lg, dtype=np.float32), np.float32(0.01))
    qscale = np.float32(1.0) / (np.sqrt(np.float32(D)) * tau)

    t_i = (np.arange(T, dtype=np.float32) / np.float32(T - 1)).astype(np.float32)
    dist = np.abs(t_i[:, None] - t_i[None, :]).astype(np.float32)
    dmat = (np.exp(-gamma * dist, dtype=np.float32) + np.float32(1e-8)).astype(
        np.float32
    )

    wqT = np.ascontiguousarray((Wq * qscale).T)
    wkT = np.ascontiguousarray(Wk.T)
    wvT = np.ascontiguousarray(Wv.T)

    with_bq = bool(np.any(bq))
    with_bk = bool(np.any(bk))
    with_bv = bool(np.any(bv))

    in_maps = []
    for c in range(NCORES):
        n0 = c * NL
        m = {
            "H_i": np.ascontiguousarray(H_i[:, :, n0 : n0 + NL, :]),
            "H_j": np.ascontiguousarray(H_j[:, :, n0 : n0 + NL, :]),
            "WqT": wqT,
            "WkT": wkT,
            "WvT": wvT,
            "Dmat": dmat,
        }
        if with_bq:
            m["bq"] = np.ascontiguousarray((bq * qscale).reshape(1, D))
        if with_bk:
            m["bk"] = np.ascontiguousarray(bk.reshape(1, D))
        if with_bv:
            m["bv"] = np.ascontiguousarray(bv.reshape(1, D))
        in_maps.append(m)
    return in_maps, (with_bq, with_bk, with_bv)


def kernel(H_i, H_j, Wq, bq, Wk, bk, Wv, bv, log_gamma, log_tau, _timers=None):
    in_maps, flags = _prepare_in_maps(
        H_i, H_j, Wq, bq, Wk, bk, Wv, bv, log_gamma, log_tau
    )
    run = _get_runner(*flags)
    per_core = run(in_maps, timers=_timers)  # [NCORES, B, T, NL, D]
    full = np.concatenate([per_core[c] for c in range(NCORES)], axis=2)
    return full


# revision 10
# speedup vs baseline: 1.2864x; 1.2864x over previous
"""CrossModalTemporalAligner kernel for Trainium2 (8 NeuronCores, Bass/Tile).

Math (per batch b, node n):
    Q = H_i[b,:,n,:] @ Wq.T + bq            [Ti, d]
    K = H_j[b,:,n,:] @ Wk.T + bk            [Tj, d]
    V = H_j[b,:,n,:] @ Wv.T + bv            [Tj, d]
    S = Q @ K.T / (sqrt(d) * tau)           [Ti, Tj]
    P = softmax(S + log(exp(-gamma*dist) + 1e-8), axis=-1)
    O = P @ V                               [Ti, d]

Device strategy: data-parallel over the node axis (64 nodes -> 8 nodes/core),
each (b, n) pair fully independent.  The softmax+decay is computed in the
equivalent multiplicative form  P ~ exp(S) * (exp(-gamma*dist)+1e-8)  and
normalized by the row sum (scores are O(6) for these inputs so max-free exp is
safe in fp32).  1/(sqrt(d)*tau) is folded into Wq host-side; the decay factor
matrix is precomputed host-side from the scalar log_gamma.

On-device layout per pair (P=128 partitions, matmuls in f32r):
    X_i, X_j loaded naturally [t, d], transposed on PE -> XiT/XjT [d, t]
    S is computed as X_i M X_j.T with M = Wq.T Wk / (sqrt(d) tau) folded on
    the host, which removes the Q projection entirely:
    GT[oc] = M XjT        [d, s]     lhsT = M.T chunk (host-precomputed)
    V[sc]  = X_j Wv.T     [s, dv]    lhsT = XjT block
    ST[sc] = GT.T-contracted with XiT -> [s_block, t]  (= S transposed)
    PT[sc] = exp(ST) * Dmat[sc]      (ACT exp, DVE multiply)
    rowsum[1, t] += ones.T @ PT[sc]  (PE), transposed to [t,1] blocks on PE,
    reciprocal on DVE; O[tb] = PT-contracted with V, scaled by recip at evict.
"""

import time

import numpy as np

B, T, NNODES, D = 4, 512, 64, 512
NCORES = 8
NL = NNODES // NCORES  # nodes per core
P = 128
C4 = 4  # 512 / 128

_CACHE = {}


def _build_program(with_bq, with_bk, with_bv):
    import concourse.bass as bass
    import concourse.mybir as mybir
    from concourse import bacc
    from concourse.bass import ts
    from concourse.masks import make_identity
    from concourse.tile import TileContext

    f32 = mybir.dt.float32
    f32r = mybir.dt.float32r  # rounded fp32: 1 cycle/row on PE vs 4 for fp32
    AF = mybir.ActivationFunctionType
    ALU = mybir.AluOpType

    # With zero q/k biases S = X_i M X_j.T with M = Wq'.T Wk folded host-side,
    # removing the Q projection.  With q/k biases fall back to separate Q/K.
    fused = not (with_bq or with_bk)

    nc = bacc.Bacc(
        "TRN2", num_devices=NCORES, debug=False, target_bir_lowering=False
    )
    hi = nc.dram_tensor("H_i", [B, T, NL, D], f32, kind="ExternalInput").ap()
    hj = nc.dram_tensor("H_j", [B, T, NL, D], f32, kind="ExternalInput").ap()
    if fused:
        mtd = nc.dram_tensor("MT", [D, D], f32, kind="ExternalInput").ap()
    else:
        wqT = nc.dram_tensor("WqT", [D, D], f32, kind="ExternalInput").ap()
        wkT = nc.dram_tensor("WkT", [D, D], f32, kind="ExternalInput").ap()
    wvT = nc.dram_tensor("WvT", [D, D], f32, kind="ExternalInput").ap()
    dmat = nc.dram_tensor("Dmat", [T, T], f32, kind="ExternalInput").ap()
    bq = bk = bv = None
    if with_bq:
        bq = nc.dram_tensor("bq", [1, D], f32, kind="ExternalInput").ap()
    if with_bk:
        bk = nc.dram_tensor("bk", [1, D], f32, kind="ExternalInput").ap()
    if with_bv:
        bv = nc.dram_tensor("bv", [1, D], f32, kind="ExternalInput").ap()
    out = nc.dram_tensor("Out", [B, T, NL, D], f32, kind="ExternalOutput").ap()

    with TileContext(nc) as tc:
        with (
            tc.tile_pool(name="const", bufs=1) as cpool,
            tc.tile_pool(name="xin", bufs=2) as xpool,
            tc.tile_pool(name="xt", bufs=2) as xtpool,
            tc.tile_pool(name="proj", bufs=2) as projpool,
            tc.tile_pool(name="pmat", bufs=2) as ppool,
            tc.tile_pool(name="outs", bufs=3) as opool,
            tc.tile_pool(name="small", bufs=2) as spool,
            tc.tile_pool(name="psum", bufs=6, space="PSUM") as psum,
            tc.tile_pool(name="psum_s", bufs=2, space="PSUM") as psum_s,
        ):
            # ---- constants (weights rounded to f32r once at startup) ----
            wstage = cpool.tile([P, C4, D], f32, name="wstage")
            if fused:
                mt_sb = cpool.tile([P, C4, D], f32r, name="mt_sb")
                nc.sync.dma_start(
                    out=wstage[:], in_=mtd.rearrange("(c p) n -> p c n", p=P)
                )
                nc.vector.tensor_copy(mt_sb[:], wstage[:])
            else:
                wq_sb = cpool.tile([P, C4, D], f32r, name="wq_sb")
                nc.sync.dma_start(
                    out=wstage[:], in_=wqT.rearrange("(c p) n -> p c n", p=P)
                )
                nc.vector.tensor_copy(wq_sb[:], wstage[:])
                wk_sb = cpool.tile([P, C4, D], f32r, name="wk_sb")
                nc.sync.dma_start(
                    out=wstage[:], in_=wkT.rearrange("(c p) n -> p c n", p=P)
                )
                nc.vector.tensor_copy(wk_sb[:], wstage[:])
            wv_sb = cpool.tile([P, C4, D], f32r, name="wv_sb")
            nc.sync.dma_start(out=wstage[:], in_=wvT.rearrange("(c p) n -> p c n", p=P))
            nc.vector.tensor_copy(wv_sb[:], wstage[:])
            dm_sb = cpool.tile([P, C4, T], f32, name="dm_sb")
            nc.sync.dma_start(out=dm_sb[:], in_=dmat.rearrange("(c p) n -> p c n", p=P))
            identity = cpool.tile([P, P], f32, name="identity")
            make_identity(nc, identity[:])
            ones_f32 = cpool.tile([P, 1], f32, name="ones_f32")
            nc.gpsimd.memset(ones_f32[:], 1.0)
            ones_col = cpool.tile([P, 1], f32r, name="ones_col")
            nc.vector.tensor_copy(ones_col[:], ones_f32[:])
            ones_row = None
            if with_bq or with_bk or with_bv:
                ones_row = cpool.tile([1, T], f32, name="ones_row")
                nc.gpsimd.memset(ones_row[:], 1.0)
            bq_sb = bk_sb = bv_sb = None
            if with_bq:
                bq_sb = cpool.tile([1, D], f32, name="bq_sb")
                nc.sync.dma_start(out=bq_sb[:], in_=bq[:])
            if with_bk:
                bk_sb = cpool.tile([1, D], f32, name="bk_sb")
                nc.sync.dma_start(out=bk_sb[:], in_=bk[:])
            if with_bv:
                bv_sb = cpool.tile([1, D], f32, name="bv_sb")
                nc.sync.dma_start(out=bv_sb[:], in_=bv[:])

            for b in range(B):
                for nl in range(NL):
                    # ---- load activations, naturally [t, d] ----
                    xi = xpool.tile([P, C4, D], f32, tag="xi", name="xi")
                    nc.sync.dma_start(
                        out=xi[:], in_=hi[b, :, nl, :].rearrange("(c p) d -> p c d", p=P)
                    )
                    xj = xpool.tile([P, C4, D], f32, tag="xj", name="xj")
                    nc.sync.dma_start(
                        out=xj[:], in_=hj[b, :, nl, :].rearrange("(c p) d -> p c d", p=P)
                    )

                    # ---- transpose to [d, t] on PE ----
                    xiT = xtpool.tile([P, C4, T], f32r, tag="xiT", name="xiT")
                    xjT = xtpool.tile([P, C4, T], f32r, tag="xjT", name="xjT")
                    for src, dst, on_act in ((xi, xiT, True), (xj, xjT, False)):
                        for dc in range(C4):
                            pt = psum.tile([P, T], f32, tag="mm", name="pt")
                            for tcc in range(C4):
                                nc.tensor.transpose(
                                    pt[:, ts(tcc, P)], src[:, tcc, ts(dc, P)], identity[:]
                                )
                            if on_act:
                                nc.scalar.copy(dst[:, dc, :], pt[:])
                            else:
                                nc.vector.tensor_copy(dst[:, dc, :], pt[:])

                    # ---- projections ----
                    if fused:
                        # GT[d_block, s] = M XjT ; S^T later contracts GT with XiT
                        gT = projpool.tile([P, C4, T], f32r, tag="gT", name="gT")
                        for oc in range(C4):
                            pg = psum.tile([P, T], f32, tag="mm", name="pg")
                            for kc in range(C4):
                                nc.tensor.matmul(
                                    pg[:],
                                    mt_sb[:, kc, ts(oc, P)],
                                    xjT[:, kc, :],
                                    start=(kc == 0),
                                    stop=(kc == 3),
                                )
                            nc.scalar.copy(gT[:, oc, :], pg[:])
                    else:
                        qT = projpool.tile([P, C4, T], f32r, tag="qT", name="qT")
                        for oc in range(C4):
                            pq = psum.tile([P, T], f32, tag="mm", name="pq")
                            for kc in range(C4):
                                nc.tensor.matmul(
                                    pq[:],
                                    wq_sb[:, kc, ts(oc, P)],
                                    xiT[:, kc, :],
                                    start=(kc == 0),
                                    stop=(kc == 3 and not with_bq),
                                )
                            if with_bq:
                                nc.tensor.matmul(
                                    pq[:], bq_sb[0:1, ts(oc, P)], ones_row[0:1, :],
                                    start=False, stop=True,
                                )
                            nc.scalar.copy(qT[:, oc, :], pq[:])

                        kT = projpool.tile([P, C4, T], f32r, tag="kT", name="kT")
                        for oc in range(C4):
                            pk = psum.tile([P, T], f32, tag="mm", name="pk")
                            for kc in range(C4):
                                nc.tensor.matmul(
                                    pk[:],
                                    wk_sb[:, kc, ts(oc, P)],
                                    xjT[:, kc, :],
                                    start=(kc == 0),
                                    stop=(kc == 3 and not with_bk),
                                )
                            if with_bk:
                                nc.tensor.matmul(
                                    pk[:], bk_sb[0:1, ts(oc, P)], ones_row[0:1, :],
                                    start=False, stop=True,
                                )
                            nc.scalar.copy(kT[:, oc, :], pk[:])

                    vm = projpool.tile([P, C4, D], f32r, tag="vm", name="vm")
                    for sc in range(C4):
                        pv = psum.tile([P, D], f32, tag="mm", name="pv")
                        for kc in range(C4):
                            nc.tensor.matmul(
                                pv[:],
                                xjT[:, kc, ts(sc, P)],
                                wv_sb[:, kc, :],
                                start=(kc == 0),
                                stop=(kc == 3 and not with_bv),
                            )
                        if with_bv:
                            nc.tensor.matmul(
                                pv[:], ones_row[0:1, 0:P], bv_sb[0:1, :],
                                start=False, stop=True,
                            )
                        nc.vector.tensor_copy(vm[:, sc, :], pv[:])

                    # ---- S^T per s-block, multiplicative-decay softmax ----
                    pm = ppool.tile([P, C4, T], f32r, tag="pm", name="pm")
                    prow = psum_s.tile([1, T], f32, tag="sm", name="prow")
                    for sc in range(C4):
                        ps = psum.tile([P, T], f32, tag="mm", name="ps")
                        for qc in range(C4):
                            nc.tensor.matmul(
                                ps[:],
                                gT[:, qc, ts(sc, P)] if fused
                                else kT[:, qc, ts(sc, P)],
                                xiT[:, qc, :] if fused else qT[:, qc, :],
                                start=(qc == 0),
                                stop=(qc == 3),
                            )
                        nc.scalar.activation(pm[:, sc, :], ps[:], AF.Exp)
                        nc.vector.tensor_tensor(
                            pm[:, sc, :], pm[:, sc, :], dm_sb[:, sc, :], ALU.mult
                        )
                        nc.tensor.matmul(
                            prow[:], ones_col[:], pm[:, sc, :],
                            start=(sc == 0), stop=(sc == 3),
                        )

                    rsum_row = spool.tile([1, T], f32, tag="rsr", name="rsum_row")
                    nc.scalar.copy(rsum_row[:], prow[:])
                    rr_ps = psum_s.tile([P, C4], f32, tag="sm", name="rr_ps")
                    for tb in range(C4):
                        nc.tensor.transpose(
                            rr_ps[:, tb : tb + 1],
                            rsum_row[0:1, ts(tb, P)],
                            identity[0:1, 0:1],
                        )
                    rr_col = spool.tile([P, C4], f32, tag="rrc", name="rr_col")
                    nc.vector.reciprocal(rr_col[:], rr_ps[:])

                    # ---- O = P V, normalized at eviction ----
                    for tb in range(C4):
                        po = psum.tile([P, D], f32, tag="mm", name="po")
                        for sc in range(C4):
                            nc.tensor.matmul(
                                po[:],
                                pm[:, sc, ts(tb, P)],
                                vm[:, sc, :],
                                start=(sc == 0),
                                stop=(sc == 3),
                            )
                        ob = opool.tile([P, D], f32, tag="ob", name="ob")
                        nc.vector.tensor_scalar_mul(ob[:], po[:], rr_col[:, tb : tb + 1])
                        nc.sync.dma_start(
                            out=out[b, ts(tb, P), nl, :], in_=ob[:]
                        )

    nc.finalize()
    return nc


def _get_runner(with_bq, with_bk, with_bv):
    """Build (once) the Bass program and a jit-compiled 8-core executor.

    Replicates the multi-core body of concourse.bass2jax.run_bass_via_pjrt so
    the jax.jit executable is cached across calls (run_bass_via_pjrt builds a
    fresh jit per invocation).
    """
    key = (with_bq, with_bk, with_bv)
    if key in _CACHE:
        return _CACHE[key]

    import jax
    import concourse.mybir as mybir
    from concourse import bass2jax
    from jax.sharding import Mesh, PartitionSpec
    from jax.experimental.shard_map import shard_map

    nc = _build_program(with_bq, with_bk, with_bv)
    bass2jax.install_neuronx_cc_hook()

    partition_name = nc.partition_id_tensor.name if nc.partition_id_tensor else None
    in_names, out_names, out_avals, zero_outs = [], [], [], []
    for alloc in nc.m.functions[0].allocations:
        if not isinstance(alloc, mybir.MemoryLocationSet):
            continue
        name = alloc.memorylocations[0].name
        if alloc.kind == "ExternalInput":
            if name != partition_name:
                in_names.append(name)
        elif alloc.kind == "ExternalOutput":
            out_names.append(name)
            shape = tuple(alloc.tensor_shape)
            dtype = mybir.dt.np(alloc.dtype)
            out_avals.append(jax.core.ShapedArray(shape, dtype))
            zero_outs.append(np.zeros(shape, dtype))
    n_params = len(in_names)
    n_outs = len(out_avals)
    in_names = in_names + out_names
    if partition_name is not None:
        in_names.append(partition_name)

    donate = tuple(range(n_params, n_params + n_outs))

    def _body(*args):
        operands = list(args)
        if partition_name is not None:
            operands.append(bass2jax.partition_id_tensor())
        outs = bass2jax._bass_exec_p.bind(
            *operands,
            out_avals=tuple(out_avals),
            in_names=tuple(in_names),
            out_names=tuple(out_names),
            lowering_input_output_aliases=(),
            sim_require_finite=True,
            sim_require_nnan=True,
            nc=nc,
        )
        return tuple(outs)

    devices = jax.devices()[:NCORES]
    mesh = Mesh(np.asarray(devices), ("core",))
    in_specs = (PartitionSpec("core"),) * (n_params + n_outs)
    out_specs = (PartitionSpec("core"),) * len(out_names)
    sharded = jax.jit(
        shard_map(_body, mesh=mesh, in_specs=in_specs, out_specs=out_specs,
                  check_rep=False),
        donate_argnums=donate,
        keep_unused=True,
    )
    param_names = in_names[:n_params]

    def run(in_maps, timers=None):
        concat_in = [
            np.concatenate([np.asarray(m[name]) for m in in_maps], axis=0)
            for name in param_names
        ]
        concat_zeros = [
            np.zeros((NCORES * z.shape[0], *z.shape[1:]), z.dtype) for z in zero_outs
        ]
        if timers is not None:
            t0 = time.perf_counter()
            out_arrs = sharded(*concat_in, *concat_zeros)
            jax.block_until_ready(out_arrs)
            timers.append(time.perf_counter() - t0)
        else:
            out_arrs = sharded(*concat_in, *concat_zeros)
        full = np.asarray(out_arrs[0]).reshape(NCORES, *out_avals[0].shape)
        return full

    _CACHE[key] = run
    return run


def _prepare_in_maps(H_i, H_j, Wq, bq, Wk, bk, Wv, bv, log_gamma, log_tau):
    H_i = np.asarray(H_i, dtype=np.float32)
    H_j = np.asarray(H_j, dtype=np.float32)
    Wq = np.asarray(Wq, dtype=np.float32)
    Wk = np.asarray(Wk, dtype=np.float32)
    Wv = np.asarray(Wv, dtype=np.float32)
    bq = np.asarray(bq, dtype=np.float32)
    bk = np.asarray(bk, dtype=np.float32)
    bv = np.asarray(bv, dtype=np.float32)
    lg = np.float32(np.asarray(log_gamma))
    lt = np.float32(np.asarray(log_tau))

    tau = np.maximum(np.exp(lt, dtype=np.float32), np.float32(0.01))
    gamma = np.maximum(np.exp(lg, dtype=np.float32), np.float32(0.01))
    qscale = np.float32(1.0) / (np.sqrt(np.float32(D)) * tau)

    t_i = (np.arange(T, dtype=np.float32) / np.float32(T - 1)).astype(np.float32)
    dist = np.abs(t_i[:, None] - t_i[None, :]).astype(np.float32)
    dmat = (np.exp(-gamma * dist, dtype=np.float32) + np.float32(1e-8)).astype(
        np.float32
    )

    wvT = np.ascontiguousarray(Wv.T)

    with_bq = bool(np.any(bq))
    with_bk = bool(np.any(bk))
    with_bv = bool(np.any(bv))
    fused = not (with_bq or with_bk)

    if fused:
        # M[d,e] = sum_a Wq'[a,d] Wk[a,e];  S = X_i M X_j^T.  Device wants M^T.
        m64 = (Wq.astype(np.float64) * float(qscale)).T @ Wk.astype(np.float64)
        mT = np.ascontiguousarray(m64.T.astype(np.float32))
    else:
        wqT = np.ascontiguousarray((Wq * qscale).T)
        wkT = np.ascontiguousarray(Wk.T)

    in_maps = []
    for c in range(NCORES):
        n0 = c * NL
        m = {
            "H_i": np.ascontiguousarray(H_i[:, :, n0 : n0 + NL, :]),
            "H_j": np.ascontiguousarray(H_j[:, :, n0 : n0 + NL, :]),
            "WvT": wvT,
            "Dmat": dmat,
        }
        if fused:
            m["MT"] = mT
        else:
            m["WqT"] = wqT
            m["WkT"] = wkT
        if with_bq:
            m["bq"] = np.ascontiguousarray((bq * qscale).reshape(1, D))
        if with_bk:
            m["bk"] = np.ascontiguousarray(bk.reshape(1, D))
        if with_bv:
            m["bv"] = np.ascontiguousarray(bv.reshape(1, D))
        in_maps.append(m)
    return in_maps, (with_bq, with_bk, with_bv)


def kernel(H_i, H_j, Wq, bq, Wk, bk, Wv, bv, log_gamma, log_tau, _timers=None):
    in_maps, flags = _prepare_in_maps(
        H_i, H_j, Wq, bq, Wk, bk, Wv, bv, log_gamma, log_tau
    )
    run = _get_runner(*flags)
    per_core = run(in_maps, timers=_timers)  # [NCORES, B, T, NL, D]
    full = np.concatenate([per_core[c] for c in range(NCORES)], axis=2)
    return full
